# revision 1
# baseline (speedup 1.0000x reference)
"""Trainium2 Bass kernel for causal multi-head attention.

Problem: B=4, S=2048, D=1024, H=16 heads (d_head=64), fp32 I/O.
    qkv = x @ w_qkv + b_qkv ; causal softmax attention ; out @ w_out + b_out

Sharding over 8 NeuronCores: data-parallel over batch (4) x
tensor-parallel over head-groups (2 groups of 8 heads). Core c handles
batch c//2, head-group c%2. No collectives: each core returns its
partial out-projection y_partial = attn_out_g @ w_out[rows_g]; the host
sums the two group partials per batch and adds b_out.

Per-core layout (everything transposed so no on-device transposes):
  host passes xT [D, S];  qT/kT = w.T @ xT  (w stationary),
  v = xT.T @ w_v (natural [S, 512], with a ones column appended per
  head so the attention AV matmul also produces the softmax
  denominator);  scoresT[j, i] = kT.T @ qT per head (K=64);  exp on
  ScalarE with the 1/sqrt(d) scale folded in (max-free softmax: logits
  here are < ~7, exp is safe);  causal handled by narrowing every
  scores/exp/AV op to the valid column range [c0, IB) per j-tile plus
  one shared 128x128 triangular mask on the partial diagonal block;
  outT_h = v_aug.T @ expT accumulated over j, software-pipelined with
  the AV matmuls skewed 2 j-tiles behind their scores matmuls so the
  in-order PE queue doesn't stall on the ScalarE exp;  normalize each
  512-half by the broadcast fast-reciprocal of the denominator row as
  soon as its last AV lands;  y = attn_outT.T @ w_out (natural
  layout), emitted in two segments (queries 0-1023 right after their
  attention blocks) borrowing the scores PSUM slots -> DMA out.
"""

import sys

if "/opt/trn_rl_repo" not in sys.path:
    sys.path.insert(0, "/opt/trn_rl_repo")

import numpy as np
import ml_dtypes

B, S, D = 4, 2048, 1024
H, DH = 16, 64
G = 2                # tensor-parallel head groups
HPG = H // G         # heads per group (8)
CG = HPG * DH        # channel cols per group (512)
N_CORES = 8
BF16 = ml_dtypes.bfloat16

KT = D // 128        # 8 contraction k-tiles for the projections
IB = 1024            # i-block (query positions per attention block)
NIB = S // IB        # 2

_cache = {}


def _build_program():
    import concourse.tile as tile
    from concourse import bacc, mybir

    f32 = mybir.dt.float32
    bf16 = mybir.dt.bfloat16
    Exp = mybir.ActivationFunctionType.Exp
    Ident = mybir.ActivationFunctionType.Identity

    nc = bacc.Bacc("TRN2", target_bir_lowering=False, debug=False,
                   num_devices=N_CORES)

    xT_d = nc.dram_tensor("xT", [D, S], bf16, kind="ExternalInput").ap()
    wq_d = nc.dram_tensor("wq", [D, CG], bf16, kind="ExternalInput").ap()
    wk_d = nc.dram_tensor("wk", [D, CG], bf16, kind="ExternalInput").ap()
    wv_d = nc.dram_tensor("wv", [D, CG], bf16, kind="ExternalInput").ap()
    bq_d = nc.dram_tensor("bq", [CG // 128, 128, 1], f32, kind="ExternalInput").ap()
    bk_d = nc.dram_tensor("bk", [CG // 128, 128, 1], f32, kind="ExternalInput").ap()
    bv_d = nc.dram_tensor("bv", [1, CG], bf16, kind="ExternalInput").ap()
    wo_d = nc.dram_tensor("wo", [CG, D], bf16, kind="ExternalInput").ap()
    tri_d = nc.dram_tensor("tri", [128, 128], bf16, kind="ExternalInput").ap()
    y_d = nc.dram_tensor("y", [S, D], f32, kind="ExternalOutput").ap()

    with tile.TileContext(nc) as tc:
        with (
            tc.tile_pool(name="consts", bufs=1) as cpool,
            tc.tile_pool(name="acts", bufs=1) as apool,
            tc.tile_pool(name="exps", bufs=12) as epool,
            tc.tile_pool(name="small", bufs=6) as spool,
            tc.tile_pool(name="rbc", bufs=4) as rpool,
            tc.tile_pool(name="ystage", bufs=3) as ypool,
        ):
            # ---- load constants (wq/xT interleaved: the k-outer qk loop
            # can start after the first (wq, xT) k-tile pair lands) ----
            wq, wk, wv, xt = [], [], [], []
            for k in range(KT):
                t = cpool.tile([128, CG], bf16, tag=f"wq{k}", name=f"wq{k}")
                nc.sync.dma_start(t[:], wq_d[k * 128:(k + 1) * 128, :])
                wq.append(t)
                t = cpool.tile([128, S], bf16, tag=f"xt{k}", name=f"xt{k}")
                nc.sync.dma_start(t[:], xT_d[k * 128:(k + 1) * 128, :])
                xt.append(t)
            for name, dram, lst in (("wk", wk_d, wk), ("wv", wv_d, wv)):
                for k in range(KT):
                    t = cpool.tile([128, CG], bf16, tag=f"{name}{k}")
                    nc.sync.dma_start(t[:], dram[k * 128:(k + 1) * 128, :])
                    lst.append(t)
            bqc, bkc = [], []
            for name, dram, lst in (("bq", bq_d, bqc), ("bk", bk_d, bkc)):
                for m in range(CG // 128):
                    t = cpool.tile([128, 1], f32, tag=f"{name}{m}")
                    nc.sync.dma_start(t[:], dram[m])
                    lst.append(t)
            bv_row = cpool.tile([1, CG], bf16, tag="bv")
            nc.sync.dma_start(bv_row[:], bv_d[:])
            tri = cpool.tile([128, 128], bf16, tag="tri")
            nc.sync.dma_start(tri[:], tri_d[:])
            wo = []
            for k in range(CG // 128):
                t = cpool.tile([128, D], bf16, tag=f"wo{k}")
                nc.sync.dma_start(t[:], wo_d[k * 128:(k + 1) * 128, :])
                wo.append(t)
            ones_row = cpool.tile([1, 128], bf16, tag="ones")
            nc.gpsimd.memset(ones_row[:], 1.0)

            # ---- persistent activations ----
            qT = [apool.tile([128, S], bf16, tag=f"qT{m}", name=f"qT{m}")
                  for m in range(CG // 128)]
            kTt = [apool.tile([128, S], bf16, tag=f"kT{m}", name=f"kT{m}")
                   for m in range(CG // 128)]
            # v with a ones column per head: [s, 65*h + (0..63)] = v_h,
            # [s, 65*h + 64] = 1
            vst = [apool.tile([128, HPG * (DH + 1)], bf16, tag=f"v{m}",
                              name=f"v{m}")
                   for m in range(S // 128)]
            aoT = [apool.tile([128, S], bf16, tag=f"aoT{m}", name=f"aoT{m}")
                   for m in range(CG // 128)]

            # v ones columns: written once, disjoint from the v copies below
            for st in range(S // 128):
                for h in range(HPG):
                    nc.gpsimd.memset(
                        vst[st][:, h * (DH + 1) + DH:(h + 1) * (DH + 1)], 1.0)

            # ---- phase 1: projections ----
            with tc.tile_pool(name="psum_qkv", bufs=6, space="PSUM") as qkvp:
                # qT / kT: lhsT = w slice (stationary), rhs = xT (moving).
                # k-outer with 4 live accumulators so compute starts as soon
                # as the first (wq, xT) k-tile pair is resident; bias added
                # on ScalarE during the PSUM->SBUF copy.
                for m in range(2 * (CG // 128)):
                    wtiles, bcols, out = ((wq, bqc, qT) if m < CG // 128
                                          else (wk, bkc, kTt))
                    mi = m % (CG // 128)
                    pss = [qkvp.tile([128, 512], f32, tag="qkps",
                                     name=f"qkps{m}_{n}", bufs=6)
                           for n in range(S // 512)]
                    for k in range(KT):
                        for n in range(S // 512):
                            nc.tensor.matmul(
                                pss[n][:],
                                wtiles[k][:, mi * 128:(mi + 1) * 128],
                                xt[k][:, n * 512:(n + 1) * 512],
                                start=(k == 0), stop=(k == KT - 1))
                    for n in range(S // 512):
                        nc.scalar.activation(
                            out[mi][:, n * 512:(n + 1) * 512], pss[n][:],
                            Ident, bias=bcols[mi][:])
                # v natural: lhsT = xT slice (stationary), rhs = w_v;
                # K=1 ones x bv matmul adds the bias row.
                for st in range(S // 128):
                    ps = qkvp.tile([128, CG], f32, tag="psv", bufs=2)
                    for k in range(KT):
                        nc.tensor.matmul(
                            ps[:], xt[k][:, st * 128:(st + 1) * 128],
                            wv[k][:], start=(k == 0), stop=False)
                    nc.tensor.matmul(ps[:], ones_row[:], bv_row[:],
                                     start=False, stop=True)
                    for h in range(HPG):
                        nc.vector.tensor_copy(
                            vst[st][:, h * (DH + 1):h * (DH + 1) + DH],
                            ps[:, h * DH:(h + 1) * DH])

            # ---- phase 2: attention ----
            with (
                tc.tile_pool(name="psum_s", bufs=2, space="PSUM") as sp,
                tc.tile_pool(name="psum_av", bufs=2, space="PSUM") as avp,
            ):
                def norm_half(av, p, sub, ib, n):
                    # normalize half n of head (2p+sub) as soon as its last
                    # AV matmul has accumulated: fast reciprocal of the
                    # denominator, broadcast across partitions, scale.
                    # custom-DVE ops must not read PSUM (kills the exec
                    # unit on hw): stage the denominator row in SBUF.
                    po = DH * sub
                    lo, hi = n * 512, (n + 1) * 512
                    dn = spool.tile([1, 512], f32, tag="den", name="dn")
                    nc.vector.tensor_copy(dn[:], av[DH:DH + 1, lo:hi])
                    rc = spool.tile([1, 512], f32, tag="recip", name="rc")
                    nc.vector.reciprocal_approx_fast(rc[:], dn[:])
                    rb = rpool.tile([DH, 512], f32, tag="rbcast", name="rb")
                    nc.gpsimd.partition_broadcast(rb[:], rc[:])
                    nc.vector.tensor_mul(
                        aoT[p][po:po + DH, ib * IB + lo:ib * IB + hi],
                        av[0:DH, lo:hi], rb[:])

                def out_proj(st_range):
                    # out-projection psum borrows the scores slots (tag
                    # "ps"), so no extra PSUM banks are needed.
                    for st in st_range:
                        for n in range(D // 512):
                            ps = sp.tile([128, 512], f32, tag="ps",
                                         name=f"yps{st}_{n}")
                            for k in range(CG // 128):
                                nc.tensor.matmul(
                                    ps[:],
                                    aoT[k][:, st * 128:(st + 1) * 128],
                                    wo[k][:, n * 512:(n + 1) * 512],
                                    start=(k == 0), stop=(k == CG // 128 - 1))
                            ys = ypool.tile([128, 512], f32, tag="ys",
                                            name=f"ys{st}_{n}")
                            nc.vector.tensor_copy(ys[:], ps[:])
                            nc.sync.dma_start(
                                y_d[st * 128:(st + 1) * 128,
                                    n * 512:(n + 1) * 512], ys[:])

                # Two heads interleaved per pair, AV matmuls skewed two
                # j-tiles behind their scores matmuls: each ScalarE exp has
                # ~2 full matmul periods of slack before the in-order PE
                # queue needs its result, so ACT hiccups don't stall the PE.
                # ib-major so all of aoT[:, 0:IB] is done after the first
                # four blocks and the first half of the out-projection can
                # run before the (heavier) ib=1 blocks.
                SKEW = 2
                for ib in range(NIB):
                    for p in range(HPG // 2):
                        njt = (ib + 1) * (IB // 128)
                        dstart = njt - (IB // 128)   # first diagonal j-tile
                        # per half: last j-tile that writes it (stop flag)
                        last = [dstart + 3, njt - 1]
                        avs = []
                        pend = [[], []]
                        for sub in range(2):
                            avs.append(avp.tile([DH + 1, IB], f32, tag="av",
                                                name=f"av{p}_{ib}_{sub}"))
                        for jt in range(njt + SKEW):
                            off = jt - dstart
                            c0 = 128 * off if off > 0 else 0
                            for sub in range(2):
                                h = 2 * p + sub
                                po = DH * sub
                                vcol = slice(h * (DH + 1), (h + 1) * (DH + 1))
                                if jt < njt:
                                    ps = sp.tile([128, IB], f32, tag="ps",
                                                 name=f"ps{p}_{ib}_{jt}_{sub}")
                                    for lo, hi in _halves(c0):
                                        nc.tensor.matmul(
                                            ps[:, lo:hi],
                                            kTt[p][po:po + DH,
                                                   jt * 128:(jt + 1) * 128],
                                            qT[p][po:po + DH,
                                                  ib * IB + lo:ib * IB + hi],
                                            start=True, stop=True)
                                    et = epool.tile([128, IB], bf16,
                                                    tag="expT", name="et")
                                    nc.scalar.activation(
                                        et[:, c0:IB], ps[:, c0:IB],
                                        Exp, scale=float(DH) ** -0.5)
                                    if off >= 0:
                                        nc.vector.tensor_mul(
                                            et[:, c0:c0 + 128],
                                            et[:, c0:c0 + 128], tri[:])
                                    pend[sub].append((jt, et, c0))
                                if len(pend[sub]) > SKEW or jt >= njt:
                                    if pend[sub]:
                                        item = pend[sub].pop(0)
                                        _av(nc, avs[sub], vst, vcol,
                                            *item, last)
                                        for n in range(IB // 512):
                                            if item[0] == last[n]:
                                                norm_half(avs[sub], p,
                                                          sub, ib, n)
                    if ib == 0:
                        out_proj(range(0, S // 256))

                out_proj(range(S // 256, S // 128))

    nc.compile()
    return nc


def _halves(c0):
    # the two 512-wide PSUM-bank column ranges, narrowed to the causally
    # valid region [c0, IB)
    for n in range(IB // 512):
        lo, hi = max(n * 512, c0), (n + 1) * 512
        if lo < hi:
            yield lo, hi


def _av(nc, av, vst, vcol, jt, et, c0, last):
    for n in range(IB // 512):
        lo, hi = max(n * 512, c0), (n + 1) * 512
        if lo < hi:
            nc.tensor.matmul(
                av[:, lo:hi],
                vst[jt][:, vcol],
                et[:, lo:hi],
                start=(jt == 0), stop=(jt == last[n]))


def _shard_inputs(x, w_qkv, b_qkv, w_out):
    # keep key j (partition) <= query i (free column): upper triangle
    tri = np.triu(np.ones((128, 128))).astype(BF16)
    in_maps = []
    for c in range(N_CORES):
        b, g = c // G, c % G
        sl = slice(g * CG, (g + 1) * CG)
        in_maps.append({
            "xT": np.ascontiguousarray(x[b].T).astype(BF16),
            "wq": w_qkv[:, 0 * D:1 * D][:, sl].astype(BF16),
            "wk": w_qkv[:, 1 * D:2 * D][:, sl].astype(BF16),
            "wv": w_qkv[:, 2 * D:3 * D][:, sl].astype(BF16),
            "bq": b_qkv[0 * D:1 * D][sl].reshape(CG // 128, 128, 1)
                  .astype(np.float32),
            "bk": b_qkv[1 * D:2 * D][sl].reshape(CG // 128, 128, 1)
                  .astype(np.float32),
            "bv": b_qkv[2 * D:3 * D][sl].reshape(1, CG).astype(BF16),
            "wo": w_out[sl, :].astype(BF16),
            "tri": tri,
        })
    return in_maps


def kernel(x, w_qkv, b_qkv, w_out, b_out):
    from concourse.bass_utils import run_bass_kernel_spmd

    x = np.asarray(x, np.float32)
    w_qkv = np.asarray(w_qkv, np.float32)
    b_qkv = np.asarray(b_qkv, np.float32)
    w_out = np.asarray(w_out, np.float32)
    b_out = np.asarray(b_out, np.float32)

    if "nc" not in _cache:
        _cache["nc"] = _build_program()
    nc = _cache["nc"]

    in_maps = _shard_inputs(x, w_qkv, b_qkv, w_out)
    res = run_bass_kernel_spmd(nc, in_maps, core_ids=list(range(N_CORES)))
    _cache["last_result"] = res

    y = np.empty((B, S, D), np.float32)
    for b in range(B):
        y[b] = res.results[G * b]["y"] + res.results[G * b + 1]["y"] + b_out
    return y

